# revision 1
# baseline (speedup 1.0000x reference)
"""CBOW model (embedding gather -> mean -> logits -> softmax) on 8 Trainium2
NeuronCores.

Sharding strategy (model/vocab parallel, per the hint):
  - W2 is sharded along the vocab axis: core m owns W2[:, m*12500:(m+1)*12500]
    and produces the logits/softmax column block [2048, 12500].
  - The embedding gather is sharded along batch: core m gathers the W1 rows
    for its 256 batch rows (2560 indirect-DMA row fetches), computes its
    hidden slice [256, 128], transposes it on PE, and an AllGather gives every
    core the full transposed hidden [128, 2048].
  - Softmax: pass 1 computes logits (written out) and per-row local exp-sums
    (fused into the Exp activation via accum_out); an AllReduce(add) of the
    [2048] local sums gives the global denominator; pass 2 recomputes the
    matmul tiles and applies exp(logit - log(sum)) via the Exp activation's
    per-partition bias, writing the softmax block.  Max-subtraction is not
    needed: logits ~ N(0, 12.8), |logit| < 30 always, exp() is safe in f32.
"""

import numpy as np

import concourse.bass as bass
import concourse.mybir as mybir
import concourse.tile as tile
from concourse import bacc
from concourse.masks import make_identity
import concourse.bass_utils as bass_utils

# Problem shape (hardcoded; matches reference.setup_inputs()).
V = 100000      # vocab
D = 128         # embed dim
B = 2048        # batch
C = 10          # context positions
M = 8           # cores
S = V // M      # vocab shard per core = 12500
BL = B // M     # batch rows per core for the gather = 256
P = 128         # partitions
BT = B // P     # batch tiles = 16
MMN = 512       # max moving free dim per f32 matmul (one PSUM bank)
GRP = 2048      # vocab columns per PSUM group (4 banks)

F32 = mybir.dt.float32
I32 = mybir.dt.int32
AF = mybir.ActivationFunctionType


def _groups():
    """(start, width) vocab-column groups per core; width <= GRP."""
    out = []
    g0 = 0
    while g0 < S:
        out.append((g0, min(GRP, S - g0)))
        g0 += GRP
    return out


def build_nc(n_cores: int = M, dbg: bool = False, rep: int = 1,
             timing_mode: bool = False):
    nc = bacc.Bacc("TRN2", target_bir_lowering=False, debug=False,
                   num_devices=n_cores)

    w1 = nc.dram_tensor("w1", [V, D], F32, kind="ExternalInput")
    w2s = nc.dram_tensor("w2s", [P, S], F32, kind="ExternalInput")
    idxs = nc.dram_tensor("idxs", [P, 2 * C], I32, kind="ExternalInput")
    if timing_mode:
        # same DMA traffic, but to internal DRAM scratch; tiny real output
        logits_s = nc.dram_tensor("logits_scr", [B, S], F32, kind="Internal")
        soft_s = nc.dram_tensor("soft_scr", [B, S], F32, kind="Internal")
        tiny = nc.dram_tensor("tiny", [P, 1], F32, kind="ExternalOutput")
    else:
        logits_s = nc.dram_tensor("logits_s", [B, S], F32, kind="ExternalOutput")
        soft_s = nc.dram_tensor("soft_s", [B, S], F32, kind="ExternalOutput")
    if dbg:
        hidT_d = nc.dram_tensor("hidT_d", [P, B], F32, kind="ExternalOutput")
        lsum_d = nc.dram_tensor("lsum_d", [P, BT], F32, kind="ExternalOutput")
        gsum_d = nc.dram_tensor("gsum_d", [P, BT], F32, kind="ExternalOutput")

    groups = _groups()
    rg = [list(range(n_cores))]

    with tile.TileContext(nc) as tc:
        with tc.tile_pool(name="sbuf", bufs=1) as sbuf, \
             tc.tile_pool(name="stag", bufs=2) as stagp, \
             tc.tile_pool(name="escr", bufs=2) as escrp, \
             tc.tile_pool(name="psum", bufs=2, space="PSUM") as psum, \
             tc.tile_pool(name="dram", bufs=1, space="DRAM") as dram:
          for _rep in range(rep):
            # ---- Phase A: gather + hidden slice + transpose + AllGather ----
            idx_sb = sbuf.tile([P, 2 * C], I32)
            nc.sync.dma_start(out=idx_sb[:], in_=idxs[:])

            ident = sbuf.tile([P, P], F32)
            make_identity(nc, ident[:])

            # W2 shard resident in SBUF for both passes.
            w2_sb = sbuf.tile([P, S], F32)
            nc.sync.dma_start(out=w2_sb[:], in_=w2s[:])

            hidT_loc = sbuf.tile([P, 2 * P], F32)  # [D, 256] local slice
            for h in range(2):
                gath = sbuf.tile([P, C * D], F32, tag="gath", bufs=2)
                for c in range(C):
                    j = h * C + c
                    nc.gpsimd.indirect_dma_start(
                        out=gath[:, c * D:(c + 1) * D],
                        out_offset=None,
                        in_=w1[:],
                        in_offset=bass.IndirectOffsetOnAxis(
                            ap=idx_sb[:, j:j + 1], axis=0),
                    )
                hid = sbuf.tile([P, D], F32, tag="hid", bufs=2)
                nc.vector.tensor_reduce(
                    out=hid[:],
                    in_=gath[:].rearrange("p (c d) -> p d c", c=C),
                    axis=mybir.AxisListType.X,
                    op=mybir.AluOpType.add,
                )
                tp = psum.tile([P, GRP], F32, tag="mm")
                nc.tensor.transpose(out=tp[:, :P], in_=hid[:], identity=ident[:])
                # mean over context folded in here (x 1/10)
                nc.vector.tensor_scalar_mul(
                    hidT_loc[:, h * P:(h + 1) * P], tp[:, :P], 1.0 / C)

            cc_h_in = dram.tile([P, 2 * P], F32)
            cc_h_out = dram.tile(
                [n_cores, P, 2 * P], F32,
                addr_space="Shared" if n_cores > 1 else "Local")
            nc.gpsimd.dma_start(out=cc_h_in[:], in_=hidT_loc[:])
            if n_cores > 1:
                nc.gpsimd.collective_compute(
                    "AllGather", mybir.AluOpType.bypass, replica_groups=rg,
                    ins=[cc_h_in[:]], outs=[cc_h_out[:]],
                )
            else:
                nc.gpsimd.dma_start(out=cc_h_out[0], in_=cc_h_in[:])
            hidT = sbuf.tile([P, B], F32)  # [D, 2048] full transposed hidden
            if n_cores > 1:
                nc.sync.dma_start(
                    out=hidT[:].rearrange("p (m j) -> p m j", m=n_cores),
                    in_=cc_h_out[:].rearrange("m p j -> p m j"),
                )
            else:
                # timing-only single-core variant: replicate the local slice
                for mm in range(M):
                    nc.sync.dma_start(
                        out=hidT[:, mm * 2 * P:(mm + 1) * 2 * P],
                        in_=cc_h_out[0],
                    )
            if dbg:
                nc.sync.dma_start(out=hidT_d[:], in_=hidT[:])

            # ---- Phase B: pass 1 -- logits + local exp sums ----
            lsum = sbuf.tile([P, BT], F32)
            for t in range(BT):
                lhsT = hidT[:, t * P:(t + 1) * P]
                stag = stagp.tile([P, S], F32, tag="stag")
                for gi, (g0, gw) in enumerate(groups):
                    ps = psum.tile([P, GRP], F32, tag="mm")
                    for s0 in range(0, gw, MMN):
                        w = min(MMN, gw - s0)
                        nc.tensor.matmul(
                            out=ps[:, s0:s0 + w], lhsT=lhsT,
                            rhs=w2_sb[:, g0 + s0:g0 + s0 + w],
                            start=True, stop=True)
                    nc.vector.tensor_copy(stag[:, g0:g0 + gw], ps[:, :gw])
                # exp-sums read the SBUF staging copy (not PSUM) so ACT does
                # not contend with the DVE copy on PSUM banks
                sums = sbuf.tile([P, len(groups)], F32, tag="sums", bufs=2)
                for gi, (g0, gw) in enumerate(groups):
                    escr = escrp.tile([P, GRP], F32, tag="escr")
                    nc.scalar.activation(
                        out=escr[:, :gw], in_=stag[:, g0:g0 + gw], func=AF.Exp,
                        accum_out=sums[:, gi:gi + 1])
                nc.vector.tensor_reduce(
                    out=lsum[:, t:t + 1], in_=sums[:],
                    axis=mybir.AxisListType.X, op=mybir.AluOpType.add)
                nc.sync.dma_start(
                    out=logits_s[t * P:(t + 1) * P, :], in_=stag[:])

            # ---- Phase C: AllReduce the local sums; bias = -ln(sum) ----
            cc_s_in = dram.tile([P, BT], F32)
            cc_s_out = dram.tile(
                [P, BT], F32,
                addr_space="Shared" if n_cores > 1 else "Local")
            nc.gpsimd.dma_start(out=cc_s_in[:], in_=lsum[:])
            if n_cores > 1:
                nc.gpsimd.collective_compute(
                    "AllReduce", mybir.AluOpType.add, replica_groups=rg,
                    ins=[cc_s_in[:]], outs=[cc_s_out[:]],
                )
            else:
                nc.gpsimd.dma_start(out=cc_s_out[:], in_=cc_s_in[:])
            gsum = sbuf.tile([P, BT], F32)
            nc.sync.dma_start(out=gsum[:], in_=cc_s_out[:])
            if dbg:
                nc.sync.dma_start(out=lsum_d[:], in_=lsum[:])
                nc.sync.dma_start(out=gsum_d[:], in_=gsum[:])
            nlogs = sbuf.tile([P, BT], F32)
            nc.scalar.activation(out=nlogs[:], in_=gsum[:], func=AF.Ln)
            nc.vector.tensor_scalar_mul(nlogs[:], nlogs[:], -1.0)

            # ---- Phase D: pass 2 -- softmax = exp(0.1*raw - ln(sum)) ----
            for t in range(BT):
                lhsT = hidT[:, t * P:(t + 1) * P]
                stag2 = stagp.tile([P, S], F32, tag="stag")
                for gi, (g0, gw) in enumerate(groups):
                    ps = psum.tile([P, GRP], F32, tag="mm")
                    for s0 in range(0, gw, MMN):
                        w = min(MMN, gw - s0)
                        nc.tensor.matmul(
                            out=ps[:, s0:s0 + w], lhsT=lhsT,
                            rhs=w2_sb[:, g0 + s0:g0 + s0 + w],
                            start=True, stop=True)
                    nc.scalar.activation(
                        out=stag2[:, g0:g0 + gw], in_=ps[:, :gw], func=AF.Exp,
                        bias=nlogs[:, t:t + 1])
                nc.sync.dma_start(
                    out=soft_s[t * P:(t + 1) * P, :], in_=stag2[:])

          if timing_mode:
            # force a data dependency on both scratch outputs
            ta = sbuf.tile([P, 1], F32)
            tb = sbuf.tile([P, 1], F32)
            nc.sync.dma_start(out=ta[:], in_=logits_s[0:P, 0:1])
            nc.sync.dma_start(out=tb[:], in_=soft_s[0:P, 0:1])
            nc.vector.tensor_add(ta[:], ta[:], tb[:])
            nc.sync.dma_start(out=tiny[:], in_=ta[:])

    nc.compile()
    return nc


def make_in_maps(inputs: np.ndarray, W1: np.ndarray, W2: np.ndarray,
                 n_cores: int = M):
    inputs = np.asarray(inputs).astype(np.int32)
    W1 = np.ascontiguousarray(np.asarray(W1, dtype=np.float32))
    W2 = np.asarray(W2, dtype=np.float32)
    in_maps = []
    for m in range(n_cores):
        idx_m = inputs[m * BL:(m + 1) * BL].reshape(2, P, C)
        idx_m = np.ascontiguousarray(idx_m.transpose(1, 0, 2).reshape(P, 2 * C))
        w2_m = np.ascontiguousarray(W2[:, m * S:(m + 1) * S])
        in_maps.append({"w1": W1, "w2s": w2_m, "idxs": idx_m})
    return in_maps


_NC_CACHE = {}


def kernel(inputs: np.ndarray, W1: np.ndarray, W2: np.ndarray):
    if "nc" not in _NC_CACHE:
        _NC_CACHE["nc"] = build_nc(M)
    nc = _NC_CACHE["nc"]
    in_maps = make_in_maps(inputs, W1, W2, M)
    res = bass_utils.run_bass_kernel_spmd(nc, in_maps, core_ids=list(range(M)))
    logits = np.concatenate([res.results[m]["logits_s"] for m in range(M)], axis=1)
    soft = np.concatenate([res.results[m]["soft_s"] for m in range(M)], axis=1)
    return logits, soft



# revision 6
# speedup vs baseline: 1.0559x; 1.0559x over previous
"""CBOW model (embedding gather -> mean -> logits -> softmax) on 8 Trainium2
NeuronCores.

Sharding strategy (model/vocab parallel, per the hint):
  - W1 and W2 are both sharded along the vocab axis: core m owns W1 rows
    [m*12500, (m+1)*12500) (fp16, plus an appended zero row) and W2 columns
    [m*12500, (m+1)*12500) (fp16, padded to 12544 with zero columns).  Inputs
    shipped per core are ~6.5 MB instead of a replicated ~57.6 MB.
  - Gather: every core looks up ALL 2048x10 indices against its own W1 shard;
    out-of-shard indices are remapped (host-side) to the zero row, so the
    per-core context sums are partial sums.  Chunked AllReduce(add) over the
    transposed [128, 2048] fp16 partial hidden gives every core the full
    hidden state (context mean folded in before the transpose); chunking lets
    later gathers overlap earlier logit tiles.
  - Softmax: pass 1 computes the logit shard (written out as bf16) and
    per-row local exp-sums (fused into the Exp activation via accum_out); an
    AllReduce(add) of the [2048] local sums gives the global denominator;
    pass 2 recomputes the matmul tiles and applies exp(logit - log(sum)) via
    the Exp activation's per-partition bias, writing the softmax block as
    bf16.  Max-subtraction is not needed: |logit| < 40 always, exp() is safe
    in f32.  Matmuls run in fp16 (1 cycle/row on the PE; fp32 needs 4).
"""

import numpy as np

import concourse.bass as bass
import concourse.mybir as mybir
import concourse.tile as tile
from concourse import bacc
from concourse.masks import make_identity
import concourse.bass_utils as bass_utils

# Problem shape (hardcoded; matches reference.setup_inputs()).
V = 100000      # vocab
D = 128         # embed dim
B = 2048        # batch
C = 10          # context positions
M = 8           # cores
S = V // M      # vocab shard per core = 12500
SP = 12544      # shard padded so every matmul chunk is >= 256 wide
P = 128         # partitions
BT = B // P     # batch tiles = 16
MMN = 512       # max moving free dim per matmul (one PSUM bank, f32)
GRP = 2048      # vocab columns per PSUM group (4 banks)

F32 = mybir.dt.float32
F16 = mybir.dt.float16
BF16 = mybir.dt.bfloat16
I32 = mybir.dt.int32
AF = mybir.ActivationFunctionType

# (start, width) vocab-column groups per core; width <= GRP.
GROUPS = [(g0, min(GRP, SP - g0)) for g0 in range(0, SP, GRP)]
# groups whose PSUM->SBUF logits copy runs on the scalar engine instead of
# DVE, to balance the two engines in pass 1
ACT_COPY_GROUPS = {len(GROUPS) - 1, len(GROUPS) - 2}
# batch tiles per hidden-AllReduce chunk: a small first chunk minimizes the
# exposed pipeline head; later gathers overlap pass-1 compute
CHUNKS = [2, 4, 5, 5]


def build_nc(n_cores: int = M):
    nc = bacc.Bacc("TRN2", target_bir_lowering=False, debug=False,
                   num_devices=n_cores)

    w1s = nc.dram_tensor("w1s", [S + 1, D], F16, kind="ExternalInput")
    w2s = nc.dram_tensor("w2s", [P, SP], F16, kind="ExternalInput")
    idxs = nc.dram_tensor("idxs", [P, BT * C], I32, kind="ExternalInput")
    logits_s = nc.dram_tensor("logits_s", [B, S], F16, kind="ExternalOutput")
    soft_s = nc.dram_tensor("soft_s", [B, S], BF16, kind="ExternalOutput")

    rg = [list(range(n_cores))]

    with tile.TileContext(nc) as tc:
        with tc.tile_pool(name="sbuf", bufs=1) as sbuf, \
             tc.tile_pool(name="gathp", bufs=3) as gathp, \
             tc.tile_pool(name="hidp", bufs=2) as hidp, \
             tc.tile_pool(name="stagp", bufs=2) as stagp, \
             tc.tile_pool(name="psum", bufs=2, space="PSUM") as psum, \
             tc.tile_pool(name="dram", bufs=1, space="DRAM") as dram:
            idx_sb = sbuf.tile([P, BT * C], I32)
            nc.sync.dma_start(out=idx_sb[:], in_=idxs[:])

            ident = sbuf.tile([P, P], F16)
            make_identity(nc, ident[:])

            # W2 shard resident in SBUF for both passes.
            w2_sb = sbuf.tile([P, SP], F16)
            nc.sync.dma_start(out=w2_sb[:], in_=w2s[:])

            # ---- Phase A: partial-hidden gather + transpose + AllReduce ----
            hidT = sbuf.tile([P, B], F16)   # [D, 2048] full transposed hidden
            t0 = 0
            for ci, ct in enumerate(CHUNKS):
                hch = hidp.tile([P, ct * P], F16, tag=f"hch{ct}")
                for tt in range(ct):
                    t = t0 + tt
                    gath = gathp.tile([P, C * D], F16, tag="gath")
                    for c in range(C):
                        j = t * C + c
                        nc.gpsimd.indirect_dma_start(
                            out=gath[:, c * D:(c + 1) * D],
                            out_offset=None,
                            in_=w1s[:],
                            in_offset=bass.IndirectOffsetOnAxis(
                                ap=idx_sb[:, j:j + 1], axis=0),
                        )
                    hid = hidp.tile([P, D], F32, tag="hid")
                    nc.vector.tensor_reduce(
                        out=hid[:],
                        in_=gath[:].rearrange("p (c d) -> p d c", c=C),
                        axis=mybir.AxisListType.X,
                        op=mybir.AluOpType.add,
                    )
                    hid16 = hidp.tile([P, D], F16, tag="hid16")
                    # context mean folded in here (x 1/10)
                    nc.vector.tensor_scalar_mul(hid16[:], hid[:], 1.0 / C)
                    tp = psum.tile([P, 2 * GRP], F16, tag="mm")
                    nc.tensor.transpose(out=tp[:, :P], in_=hid16[:],
                                        identity=ident[:])
                    nc.vector.tensor_copy(hch[:, tt * P:(tt + 1) * P],
                                          tp[:, :P])
                cc_in = dram.tile([P, ct * P], F16)
                cc_out = dram.tile(
                    [P, ct * P], F16,
                    addr_space="Shared" if n_cores > 1 else "Local")
                nc.gpsimd.dma_start(out=cc_in[:], in_=hch[:])
                if n_cores > 1:
                    nc.gpsimd.collective_compute(
                        "AllReduce", mybir.AluOpType.add, replica_groups=rg,
                        ins=[cc_in[:]], outs=[cc_out[:]],
                    )
                else:
                    nc.gpsimd.dma_start(out=cc_out[:], in_=cc_in[:])
                nc.sync.dma_start(out=hidT[:, t0 * P:(t0 + ct) * P],
                                  in_=cc_out[:])
                t0 += ct

            # ---- Phase B: pass 1 -- logits (bf16) + local exp sums ----
            lsum = sbuf.tile([P, BT], F32)
            escr = sbuf.tile([P, S], BF16)  # discarded exp output
            for t in range(BT):
                lhsT = hidT[:, t * P:(t + 1) * P]
                stag = stagp.tile([P, SP], F16, tag="stag1")
                for gi, (g0, gw) in enumerate(GROUPS):
                    ps = psum.tile([P, GRP], F32, tag="mm")
                    for s0 in range(0, gw, MMN):
                        w = min(MMN, gw - s0)
                        nc.tensor.matmul(
                            out=ps[:, s0:s0 + w], lhsT=lhsT,
                            rhs=w2_sb[:, g0 + s0:g0 + s0 + w],
                            start=True, stop=True)
                    if gi in ACT_COPY_GROUPS:
                        nc.scalar.copy(stag[:, g0:g0 + gw], ps[:, :gw])
                    else:
                        nc.vector.tensor_copy(stag[:, g0:g0 + gw], ps[:, :gw])
                nc.scalar.activation(
                    out=escr[:], in_=stag[:, :S], func=AF.Exp,
                    accum_out=lsum[:, t:t + 1])
                nc.sync.dma_start(
                    out=logits_s[t * P:(t + 1) * P, :], in_=stag[:, :S])

            # ---- Phase C: AllReduce the local sums; bias = -ln(sum) ----
            cc_s_in = dram.tile([P, BT], F32)
            cc_s_out = dram.tile(
                [P, BT], F32, addr_space="Shared" if n_cores > 1 else "Local")
            nc.gpsimd.dma_start(out=cc_s_in[:], in_=lsum[:])
            if n_cores > 1:
                nc.gpsimd.collective_compute(
                    "AllReduce", mybir.AluOpType.add, replica_groups=rg,
                    ins=[cc_s_in[:]], outs=[cc_s_out[:]],
                )
            else:
                nc.gpsimd.dma_start(out=cc_s_out[:], in_=cc_s_in[:])
            gsum = sbuf.tile([P, BT], F32)
            nc.sync.dma_start(out=gsum[:], in_=cc_s_out[:])
            nlogs = sbuf.tile([P, BT], F32)
            nc.scalar.activation(out=nlogs[:], in_=gsum[:], func=AF.Ln)
            nc.vector.tensor_scalar_mul(nlogs[:], nlogs[:], -1.0)

            # ---- Phase D: pass 2 -- softmax = exp(raw - ln(sum)), bf16 ----
            for t in range(BT):
                lhsT = hidT[:, t * P:(t + 1) * P]
                stag2 = stagp.tile([P, SP], BF16, tag="stag2")
                for gi, (g0, gw) in enumerate(GROUPS):
                    ps = psum.tile([P, GRP], F32, tag="mm")
                    for s0 in range(0, gw, MMN):
                        w = min(MMN, gw - s0)
                        nc.tensor.matmul(
                            out=ps[:, s0:s0 + w], lhsT=lhsT,
                            rhs=w2_sb[:, g0 + s0:g0 + s0 + w],
                            start=True, stop=True)
                    nc.scalar.activation(
                        out=stag2[:, g0:g0 + gw], in_=ps[:, :gw], func=AF.Exp,
                        bias=nlogs[:, t:t + 1])
                nc.sync.dma_start(
                    out=soft_s[t * P:(t + 1) * P, :], in_=stag2[:, :S])

    nc.compile()
    return nc


def make_in_maps(inputs: np.ndarray, W1: np.ndarray, W2: np.ndarray,
                 n_cores: int = M):
    idx = np.asarray(inputs).astype(np.int64)
    W1 = np.asarray(W1, dtype=np.float32)
    W2 = np.asarray(W2, dtype=np.float32)
    in_maps = []
    for m in range(n_cores):
        lo = m * S
        loc = idx - lo
        idxm = np.where((loc >= 0) & (loc < S), loc, S).astype(np.int32)
        idxm = np.ascontiguousarray(
            idxm.reshape(BT, P, C).transpose(1, 0, 2).reshape(P, BT * C))
        w1m = np.empty((S + 1, D), np.float16)
        w1m[:S] = W1[lo:lo + S]
        w1m[S] = 0
        w2m = np.zeros((P, SP), np.float16)
        w2m[:, :S] = W2[:, lo:lo + S]
        in_maps.append({"w1s": w1m, "w2s": w2m, "idxs": idxm})
    return in_maps


_NC_CACHE = {}


def kernel(inputs: np.ndarray, W1: np.ndarray, W2: np.ndarray):
    if "nc" not in _NC_CACHE:
        _NC_CACHE["nc"] = build_nc(M)
    nc = _NC_CACHE["nc"]
    in_maps = make_in_maps(inputs, W1, W2, M)
    res = bass_utils.run_bass_kernel_spmd(nc, in_maps, core_ids=list(range(M)))
    logits = np.empty((B, V), np.float32)
    soft = np.empty((B, V), np.float32)
    for m in range(M):
        logits[:, m * S:(m + 1) * S] = np.asarray(
            res.results[m]["logits_s"]).astype(np.float32)
        soft[:, m * S:(m + 1) * S] = np.asarray(
            res.results[m]["soft_s"]).astype(np.float32)
    return logits, soft
